# revision 5
# baseline (speedup 1.0000x reference)
"""Trainium2 Bass kernel for nn_ApproxCompressor.

Reference semantics (per sample n):
    alpha   = sigmoid(z_alpha[n])
    h[k]    = (1-alpha) * alpha^k, k < 16384          (exponential FIR)
    energy  = mean_c x[n,c,:]^2
    env     = causal_conv(energy, h)[:L]              (reference: FFT conv)
    LG      = log(env + 1e-5)
    d       = LG - (log_threshold - 6)
    W       = exp(log_knee);  c = 1/(1+exp(log_ratio)) - 1
    log_gain = 0 (d<-W) | c*(d+W)^2/(4W) (|d|<W) | c*d (d>=W)
    out     = exp(log_gain)[None] * x[n]

Kernel strategy (8 cores, pure data parallel, 4 samples/core):
  * The exponential FIR == one-pole IIR y[t] = a*y[t-1] + D[t] (D = x0^2+x1^2),
    truncation tail a^16384 underflows for any plausible alpha (asserted).
    Computed with the DVE tensor_tensor_scan instruction: each of the 128
    partitions scans its own contiguous 1024-sample chunk; chunk-boundary
    carries are fixed exactly with a 128x128 lower-triangular decay matmul
    (host-precomputed per sample) followed by a second scan with per-partition
    initial states.  All scale factors (0.5*(1-alpha)) fold into the Ln scale.
  * Quadratic knee without branches or cancellation:
        A = relu(d + W); C = min(A, 2W);  log_gain = c/(4W) * C * (2A - C)
    which is algebraically identical to the reference piecewise form.
  * Per-sample scalars enter as (128,1) SBUF columns (SPMD: one graph, all
    cores), used as ACT bias/scale and tensor_scalar operands.
"""

import os
import sys

import numpy as np


def _import_concourse():
    try:
        import concourse.bass  # noqa: F401
    except ImportError:
        for p in ("/opt/trn_rl_repo", "/root/.axon_site/_ro/trn_rl_repo"):
            if os.path.isdir(p) and p not in sys.path:
                sys.path.insert(0, p)
        import concourse.bass  # noqa: F401


_import_concourse()

import concourse.bass as bass  # noqa: E402
import concourse.tile as tile  # noqa: E402
from concourse import bacc, mybir  # noqa: E402

N, C, L = 32, 2, 131072
NCORES = 8
NLOC = N // NCORES  # samples per core
P, F = 128, 1024  # L = P * F
EPS = 1e-5
K_FIR = 16384

F32 = mybir.dt.float32
BF16 = mybir.dt.bfloat16

# param columns in the prm tensor
PRM_ALPHA, PRM_LNSCALE, PRM_B1, PRM_W2, PRM_C4W, PRM_EPS, PRM_ZERO = 0, 1, 2, 3, 4, 5, 6
NPRM = 8  # padded

# engine assignment for the flexible elementwise ops (tunable)
ENG = {
    "sq0": "scalar",   # x0^2          (ACT Square)
    "sq1": "gpsimd",   # x1^2          (TT mult)
    "eadd": "vector",  # D = sq0+sq1
    "alpha": "vector", # alpha const tile build
    "scan1": "vector",
    "scan2": "vector",
    "C": "vector",     # C = min(A, 2W)
    "Z": "vector",     # Z = 2A - C
    "Q": "gpsimd",     # Q = C*Z
    "out0": "vector",
    "out1": "gpsimd",
}

TRACE_RESULT = {}  # test harness looks here after kernel() with BASS_KERNEL_TRACE=1


def _eng(nc, name):
    return getattr(nc, ENG[name])


def build_nc():
    AF = mybir.ActivationFunctionType
    OP = mybir.AluOpType

    nc = bacc.Bacc("TRN2", target_bir_lowering=False, num_devices=NCORES)
    x_ext = nc.declare_dram_parameter("x", [NLOC, C, L], F32, isOutput=False)
    prm_ext = nc.declare_dram_parameter("prm", [NLOC, P, NPRM], F32, isOutput=False)
    tri_ext = nc.declare_dram_parameter("tri", [NLOC, P, P], F32, isOutput=False)
    out_ext = nc.declare_dram_parameter("out", [NLOC, C, L], BF16, isOutput=True)

    with tile.TileContext(nc) as tc:
        with (
            tc.tile_pool(name="pio", bufs=3) as pio,
            tc.tile_pool(name="pconst", bufs=NLOC + 1) as pconst,
            tc.tile_pool(name="pwork", bufs=2) as pwork,
            tc.tile_pool(name="pps", bufs=2, space=bass.MemorySpace.PSUM) as pps,
        ):
            for s in range(NLOC):
                # ---- loads -------------------------------------------------
                prm = pconst.tile([P, NPRM], F32, tag="prm")
                nc.sync.dma_start(out=prm[:], in_=prm_ext[s])
                tri = pconst.tile([P, P], F32, tag="tri")
                nc.sync.dma_start(out=tri[:], in_=tri_ext[s])

                xt = pio.tile([P, C * F], BF16, tag="xt")
                # f32 -> bf16 cast during DMA (SWDGE)
                nc.gpsimd.dma_start(
                    out=xt[:].rearrange("p (c f) -> p c f", c=C),
                    in_=x_ext[s].rearrange("c (p f) -> p c f", p=P),
                )

                a_col = prm[:, PRM_ALPHA : PRM_ALPHA + 1]
                lnscale_col = prm[:, PRM_LNSCALE : PRM_LNSCALE + 1]
                b1_col = prm[:, PRM_B1 : PRM_B1 + 1]
                w2_col = prm[:, PRM_W2 : PRM_W2 + 1]
                c4w_col = prm[:, PRM_C4W : PRM_C4W + 1]
                eps_col = prm[:, PRM_EPS : PRM_EPS + 1]
                zero_col = prm[:, PRM_ZERO : PRM_ZERO + 1]

                # ---- energy D = x0^2 + x1^2 (scale folded into Ln) ---------
                sq0 = pwork.tile([P, F], BF16, tag="sq0")
                if ENG["sq0"] == "scalar":
                    nc.scalar.activation(sq0[:], xt[:, 0:F], AF.Square, bias=zero_col)
                else:
                    _eng(nc, "sq0").tensor_tensor(sq0[:], xt[:, 0:F], xt[:, 0:F], OP.mult)
                sq1 = pwork.tile([P, F], BF16, tag="sq1")
                if ENG["sq1"] == "scalar":
                    nc.scalar.activation(sq1[:], xt[:, F : 2 * F], AF.Square, bias=zero_col)
                else:
                    _eng(nc, "sq1").tensor_tensor(
                        sq1[:], xt[:, F : 2 * F], xt[:, F : 2 * F], OP.mult
                    )
                D = pwork.tile([P, F], BF16, tag="D")
                _eng(nc, "eadd").tensor_tensor(D[:], sq0[:], sq1[:], OP.add)

                # ---- alpha constant tile (f32: decay rate must be exact) ---
                alpha_t = pwork.tile([P, F], F32, tag="alpha_t")
                _eng(nc, "alpha").tensor_scalar(
                    alpha_t[:], D[:], 0.0, a_col, OP.mult, OP.add
                )

                # ---- IIR via chunked scans + exact carry fix ---------------
                y1 = pwork.tile([P, F], F32, tag="y1")
                _eng(nc, "scan1").tensor_tensor_scan(
                    y1[:], alpha_t[:], D[:], 0.0, OP.mult, OP.add
                )
                s_col = pps.tile([P, 1], F32, tag="s_col")
                nc.tensor.matmul(
                    s_col[:], tri[:], y1[:, F - 1 : F], start=True, stop=True
                )
                env = pwork.tile([P, F], F32, tag="env")
                _eng(nc, "scan2").tensor_tensor_scan(
                    env[:], alpha_t[:], D[:], s_col[:], OP.mult, OP.add
                )

                # ---- gain computer -----------------------------------------
                LG = pwork.tile([P, F], F32, tag="LG")
                nc.scalar.activation(LG[:], env[:], AF.Ln, bias=eps_col, scale=lnscale_col)
                A = pwork.tile([P, F], F32, tag="A")
                nc.scalar.activation(A[:], LG[:], AF.Relu, bias=b1_col)
                Ct = pwork.tile([P, F], F32, tag="Ct")
                _eng(nc, "C").tensor_scalar_min(Ct[:], A[:], w2_col)
                Z = pwork.tile([P, F], F32, tag="Z")
                _eng(nc, "Z").scalar_tensor_tensor(
                    Z[:], A[:], 2.0, Ct[:], OP.mult, OP.subtract
                )
                Q = pwork.tile([P, F], F32, tag="Q")
                _eng(nc, "Q").tensor_tensor(Q[:], Ct[:], Z[:], OP.mult)
                gain = pwork.tile([P, F], BF16, tag="gain")
                nc.scalar.activation(gain[:], Q[:], AF.Exp, bias=zero_col, scale=c4w_col)

                # ---- apply gain + store ------------------------------------
                ot = pio.tile([P, C * F], BF16, tag="ot")
                _eng(nc, "out0").tensor_tensor(
                    ot[:, 0:F], gain[:], xt[:, 0:F], OP.mult
                )
                _eng(nc, "out1").tensor_tensor(
                    ot[:, F : 2 * F], gain[:], xt[:, F : 2 * F], OP.mult
                )
                nc.sync.dma_start(
                    out=out_ext[s].rearrange("c (p f) -> p c f", p=P),
                    in_=ot[:].rearrange("p (c f) -> p c f", c=C),
                )
    nc.finalize()
    return nc


def host_params(z_alpha, log_threshold, log_ratio, log_knee):
    """Per-sample scalars + triangular carry matrices, float32 to match ref."""
    z = z_alpha.astype(np.float64).reshape(-1)
    alpha = 1.0 / (1.0 + np.exp(-z))  # sigmoid
    aK = np.exp(K_FIR * np.log(alpha))
    assert np.all(aK < 1e-6), (
        "FIR truncation tail non-negligible; kernel needs the shift correction"
    )
    T = log_threshold.astype(np.float64).reshape(-1) - 6.0
    R = 1.0 + np.exp(log_ratio.astype(np.float64).reshape(-1))
    W = np.exp(log_knee.astype(np.float64).reshape(-1))
    c = 1.0 / R - 1.0

    n = alpha.shape[0]
    prm = np.zeros((n, P, NPRM), np.float32)
    prm[:, :, PRM_ALPHA] = alpha.astype(np.float32)[:, None]
    prm[:, :, PRM_LNSCALE] = (0.5 * (1.0 - alpha)).astype(np.float32)[:, None]
    prm[:, :, PRM_B1] = (W - T).astype(np.float32)[:, None]
    prm[:, :, PRM_W2] = (2.0 * W).astype(np.float32)[:, None]
    prm[:, :, PRM_C4W] = (c / (4.0 * W)).astype(np.float32)[:, None]
    prm[:, :, PRM_EPS] = np.float32(EPS)

    # TriT[q, p] = alpha^(F*(p-1-q)) for q <= p-1 else 0   (lhsT layout)
    k = (np.arange(P)[None, :] - 1 - np.arange(P)[:, None]).astype(np.float64)
    tri = np.zeros((n, P, P), np.float32)
    for i in range(n):
        expo = F * k * np.log(alpha[i])
        m = (k >= 0) & (expo > -100.0)
        t = np.zeros((P, P))
        t[m] = np.exp(expo[m])
        tri[i] = t.astype(np.float32)
    return prm, tri


def _ensure_ntff_hook():
    """The agent image's antenv lacks axon_hooks; synthesize it so
    run_bass_kernel_spmd(trace=True) can reach the NTFF profiler."""
    import types

    try:
        from antenv.axon_hooks import get_axon_ntff_profile_hook  # noqa: F401

        return
    except ImportError:
        pass
    try:
        from trn_agent_boot.trn_boot import _ntff_profile_via_ctypes
    except ImportError:
        return
    hook = _ntff_profile_via_ctypes("/opt/axon/libaxon_pjrt.so")
    mod = types.ModuleType("antenv.axon_hooks")
    mod._hook = hook
    mod.get_axon_ntff_profile_hook = lambda: mod._hook

    def set_axon_ntff_profile_hook(h):
        mod._hook = h

    mod.set_axon_ntff_profile_hook = set_axon_ntff_profile_hook
    import antenv

    sys.modules["antenv.axon_hooks"] = mod
    antenv.axon_hooks = mod


def kernel(input_signals, z_alpha, log_threshold, log_ratio, log_knee):
    from concourse.bass_utils import run_bass_kernel_spmd

    x = np.asarray(input_signals, np.float32)
    prm, tri = host_params(
        np.asarray(z_alpha), np.asarray(log_threshold),
        np.asarray(log_ratio), np.asarray(log_knee),
    )

    nc = build_nc()
    core_ids = list(range(NCORES))
    in_maps = [
        {
            "x": np.ascontiguousarray(x[i * NLOC : (i + 1) * NLOC]),
            "prm": np.ascontiguousarray(prm[i * NLOC : (i + 1) * NLOC]),
            "tri": np.ascontiguousarray(tri[i * NLOC : (i + 1) * NLOC]),
        }
        for i in core_ids
    ]

    trace = os.environ.get("BASS_KERNEL_TRACE", "0") == "1"
    if trace:
        _ensure_ntff_hook()
    res = run_bass_kernel_spmd(nc, in_maps, core_ids, trace=trace)
    if trace:
        TRACE_RESULT["exec_time_ns"] = res.exec_time_ns
        TRACE_RESULT["results"] = res

    out = np.concatenate(
        [np.asarray(res.results[i]["out"], np.float32) for i in range(NCORES)], axis=0
    )
    return out
